# revision 13
# baseline (speedup 1.0000x reference)
"""MoE MLP (top-2 of 8 experts) on 8 Trainium2 NeuronCores.

Strategy: expert parallelism with w-stratified mixed precision. Each of
the 8 cores owns one expert. Host-side routing sorts each expert's
(token, k) pairs by routing weight w. The T_HI highest-w pairs per
expert run a dense bf16 pipeline (w folded into the activations); the
remaining low-w pairs run a pure-fp8(e4m3) pipeline using DoubleRow
matmuls (256-deep contraction per instruction = 2x bf16 FLOP rate), w
applied on the host during combine.

Why this is safe: the kernel's error is relative to each contribution's
magnitude, and a pair's contribution is scaled by w. fp8 carries ~5.3%
relative error, so assigning only the smallest-w pairs to fp8 keeps the
global L2 error at ~sqrt(frac_w2) * 5.3% where frac_w2 is the fp8
stratum's share of sum(w^2). T_HI is chosen at runtime as the smallest
count keeping frac_w2 <= 7%, bounding total rel err ~1.5e-2 < 2e-2.

This also load-balances: every core computes exactly T_HI bf16 columns
plus its overflow (c_e - T_HI <= T_LO) fp8 columns, so per-core PE time
is nearly equal without splitting experts across cores.

Device compute per core (fp32 PSUM accumulation):
  bf16: hidT[D,Th] = relu(dw^T @ (w*x)T) ; yT[H,Th] = up^T @ hidT
  fp8:  hid8[D,Tl] = relu8(dw8^T @ x8T)  ; y8T[H,Tl] = up8^T @ hid8
"""

import os
import sys
import time

import numpy as np

for _p in ("/opt/trn_rl_repo", "/root/.axon_site/_ro/trn_rl_repo"):
    if os.path.isdir(_p) and _p not in sys.path:
        sys.path.append(_p)

import ml_dtypes

import concourse.bass as bass
import concourse.mybir as mybir
from concourse import bacc
from concourse.bass_utils import run_bass_kernel_spmd
from concourse.tile import TileContext

BF16 = ml_dtypes.bfloat16
F8 = ml_dtypes.float8_e4m3

B, S, H, E, K, D = 1, 4096, 1024, 8, 2, 2048
N = B * S
P = 128
KH = H // P    # 8 contraction tiles for the bf16 down matmul
KD = D // P    # 16 contraction tiles for the bf16 up matmul
KH2 = KH // 2  # 4 DoubleRow pairs (256-deep) for the fp8 down matmul
KD2 = KD // 2  # 8 DoubleRow pairs for the fp8 up matmul
NCORES = 8

# Max share of sum(w^2) allowed in the fp8 stratum. err ~ 5.3% *
# sqrt(FRAC) + bf16's 0.33% -> ~1.7e-2 vs the 2e-2 gate.
FRAC_W2 = 0.10

# Exposed for test harness introspection (exec_time_ns etc).
LAST_RESULT = None


def _chunks(total: int, maxc: int = 512) -> list[tuple[int, int]]:
    """(offset, size) split of `total` into <=512-wide PSUM chunks.

    The FIRST chunk is made as wide as possible: a wide chunk lowers the
    weight-stream bandwidth the k-outer matmul loop demands per cycle
    (dw bytes/cycle ~ 32768/chunk_width), which is what paces the DMA-
    starved first ~15us of the kernel. A 512+128 split beats even halves
    for 512 < total <= 640."""
    if total <= maxc:
        return [(0, total)]
    if total <= maxc + 128:
        return [(0, total - 128), (total - 128, 128)]
    n = -(-total // maxc)
    base, rem = divmod(total, n)
    out, off = [], 0
    for i in range(n):
        sz = base + (1 if i < rem else 0)
        out.append((off, sz))
        off += sz
    return out


def _r8(v: int) -> int:
    return -(-v // 8) * 8


def _build_bass(t_hi: int, t_lo: int) -> bass.Bass:
    """One expert's mixed-precision MLP. t_lo == 0 disables the fp8 path."""
    bf16 = mybir.dt.bfloat16
    f8 = mybir.dt.float8e4
    f32 = mybir.dt.float32
    DR = mybir.MatmulPerfMode.DoubleRow

    nc = bacc.Bacc()
    xT = nc.dram_tensor("xT", [H, t_hi], bf16, kind="ExternalInput")
    dw = nc.dram_tensor("dw", [H, D], bf16, kind="ExternalInput")
    up = nc.dram_tensor("up", [D, H], bf16, kind="ExternalInput")
    yT = nc.dram_tensor("yT", [H, t_hi], f32, kind="ExternalOutput")
    if t_lo:
        x8 = nc.dram_tensor("x8", [P, KH2, 2, t_lo], f8, kind="ExternalInput")
        dw8 = nc.dram_tensor("dw8", [P, KH2, 2, D], f8, kind="ExternalInput")
        up8 = nc.dram_tensor("up8", [P, KD2, 2, H], f8, kind="ExternalInput")
        y8 = nc.dram_tensor("y8", [H, t_lo], f32, kind="ExternalOutput")

    with TileContext(nc) as tc:
        with (
            tc.tile_pool(name="const", bufs=1) as const,
            tc.tile_pool(name="psum", bufs=1, space="PSUM") as psum,
            tc.tile_pool(name="outp", bufs=4) as outp,
        ):
            dw_sb = const.tile([P, KH, D], bf16)
            xT_sb = const.tile([P, KH, t_hi], bf16)
            up_sb = const.tile([P, KD, H], bf16)
            hid_sb = const.tile([P, KD, t_hi], bf16)
            if t_lo:
                dw8_sb = const.tile([P, KH2, 2, D], f8)
                x8_sb = const.tile([P, KH2, 2, t_lo], f8)
                up8_sb = const.tile([P, KD2, 2, H], f8)
                hid8_sb = const.tile([P, KD2, 2, t_lo], f8)

            # ---- DMA schedule (phases ordered by first-need time) ----
            hD = D // 2
            tchunks = _chunks(t_hi)
            n0_off, n0 = tchunks[0]
            rings = [nc.sync, nc.scalar]
            # Phase 1: dw half A + first token chunk, alternating rings
            # per k so chunk k of both mm1 operands lands together.
            for k in range(KH):
                if k == 0:
                    # Finest pieces for the very first dependencies, fanned
                    # across both rings so multiple DMA queues pull them in
                    # parallel and the first real matmul starts ASAP.
                    qD = hD // 4
                    h0 = n0 // 2
                    nc.sync.dma_start(dw_sb[:, 0, :qD], dw[:P, :qD])
                    nc.scalar.dma_start(xT_sb[:, 0, :h0], xT[:P, :h0])
                    nc.sync.dma_start(dw_sb[:, 0, qD : 2 * qD], dw[:P, qD : 2 * qD])
                    nc.scalar.dma_start(xT_sb[:, 0, h0:n0], xT[:P, h0:n0])
                    nc.sync.dma_start(dw_sb[:, 0, 2 * qD : hD], dw[:P, 2 * qD : hD])
                    continue
                rings[k % 2].dma_start(
                    dw_sb[:, k, :hD], dw[k * P : (k + 1) * P, :hD]
                )
                rings[1 - k % 2].dma_start(
                    xT_sb[:, k, :n0], xT[k * P : (k + 1) * P, :n0]
                )
                # Prefetch half-B k-chunks early so mh1 of the first
                # token chunk never waits on them.
                if k in (2, 4, 6):
                    for kb in (k - 2, k - 1):
                        rings[kb % 2].dma_start(
                            dw_sb[:, kb, hD:], dw[kb * P : (kb + 1) * P, hD:]
                        )
            # Phase 2: dw half B.
            for k in range(6, KH):
                rings[k % 2].dma_start(
                    dw_sb[:, k, hD:], dw[k * P : (k + 1) * P, hD:]
                )
            # Phase 3: remaining token chunks.
            for off, sz in tchunks[1:]:
                for k in range(KH):
                    rings[k % 2].dma_start(
                        xT_sb[:, k, off : off + sz],
                        xT[k * P : (k + 1) * P, off : off + sz],
                    )
            # Phase 4: fp8 weights + tokens (needed when mm1lo starts,
            # ~35us in).
            if t_lo:
                for kk in range(KH2):
                    rings[kk % 2].dma_start(x8_sb[:, kk], x8[:, kk])
                for kk in range(KH2):
                    rings[kk % 2].dma_start(
                        dw8_sb[:, kk, :, :hD], dw8[:, kk, :, :hD]
                    )
                    rings[1 - kk % 2].dma_start(
                        dw8_sb[:, kk, :, hD:], dw8[:, kk, :, hD:]
                    )
            # Phase 5: up weights (needed when mm2hi starts, ~47us).
            for k in range(KD):
                rings[k % 2].dma_start(up_sb[:, k, :], up[k * P : (k + 1) * P, :])
            # Phase 6: fp8 up weights (needed when mm2lo starts, ~80us).
            if t_lo:
                for kk in range(KD2):
                    rings[kk % 2].dma_start(up8_sb[:, kk], up8[:, kk])

            # ---- PE warmup (HAM un-throttles after ~3.4us of activity).
            warm_sb = const.tile([P, 640], bf16)
            nc.vector.memset(warm_sb[:], 0.0)
            warm_ps = psum.tile([P, 512], f32, tag="ps0", name="warm_ps")
            n_warm = 10
            for i in range(n_warm):
                nc.tensor.matmul(
                    warm_ps[:],
                    warm_sb[:, :P],
                    warm_sb[:, P:640],
                    start=(i == 0),
                    stop=(i == n_warm - 1),
                )

            # ---- mm1hi: hidT[D,Th] = relu(dw^T @ xT), k-outermost over
            # 8 concurrent PSUM groups.
            for n_off, n_size in tchunks:
                for mh in range(KD // 8):
                    pss = [
                        psum.tile([P, n_size], f32, tag=f"ps{m}", name=f"h{mh}_{m}")
                        for m in range(8)
                    ]
                    for k in range(KH):
                        for m in range(8):
                            md = mh * 8 + m
                            nc.tensor.matmul(
                                pss[m][:],
                                dw_sb[:, k, md * P : (md + 1) * P],
                                xT_sb[:, k, n_off : n_off + n_size],
                                start=(k == 0),
                                stop=(k == KH - 1),
                            )
                    for m in range(8):
                        md = mh * 8 + m
                        nc.vector.tensor_scalar_max(
                            hid_sb[:, md, n_off : n_off + n_size], pss[m][:], 0.0
                        )

            # ---- mm1lo (fp8 DoubleRow): hid8[D,Tl] = relu8(dw8^T@x8).
            # m-outer (weights are resident by now, no DMA pacing need):
            # each PSUM group completes after its 4 DR matmuls and its
            # relu fires immediately, so bank md%8 is free again well
            # before group md+8 needs it — no drain burst at the mh
            # boundary. relu8 lives on VECTOR: the scalar engine is
            # busy issuing DMA ring pushes until ~50us and would stall
            # the whole fp8 phase.
            if t_lo:
                for md in range(KD):
                    ps = psum.tile([P, t_lo], f32, tag=f"ps{md % 8}", name=f"lo{md}")
                    for kk in range(KH2):
                        nc.tensor.matmul(
                            ps[:],
                            dw8_sb[:, kk, :, md * P : (md + 1) * P],
                            x8_sb[:, kk],
                            start=(kk == 0),
                            stop=(kk == KH2 - 1),
                            perf_mode=DR,
                        )
                    nc.vector.tensor_scalar_max(
                        hid8_sb[:, md // 2, md % 2, :], ps[:], 0.0
                    )

            # ---- mm2hi: yT[H,Th] = up^T @ hidT.
            gi = 0
            for mh in range(H // P):
                for n_off, n_size in tchunks:
                    ps = psum.tile([P, n_size], f32, tag=f"ps{gi % 8}")
                    gi += 1
                    for k in range(KD):
                        nc.tensor.matmul(
                            ps[:],
                            up_sb[:, k, mh * P : (mh + 1) * P],
                            hid_sb[:, k, n_off : n_off + n_size],
                            start=(k == 0),
                            stop=(k == KD - 1),
                        )
                    yt = outp.tile([P, n_size], f32, tag="yt")
                    nc.vector.tensor_copy(yt[:], ps[:])
                    rings[gi % 2].dma_start(
                        yT[mh * P : (mh + 1) * P, n_off : n_off + n_size], yt[:]
                    )

            # ---- mm2lo (fp8 DoubleRow): y8T[H,Tl] = up8^T @ hid8.
            if t_lo:
                for mh in range(H // P):
                    ps = psum.tile([P, t_lo], f32, tag=f"ps{mh % 8}")
                    for kk in range(KD2):
                        nc.tensor.matmul(
                            ps[:],
                            up8_sb[:, kk, :, mh * P : (mh + 1) * P],
                            hid8_sb[:, kk],
                            start=(kk == 0),
                            stop=(kk == KD2 - 1),
                            perf_mode=DR,
                        )
                    yt = outp.tile([P, t_lo], f32, tag="yt")
                    nc.vector.tensor_copy(yt[:], ps[:])
                    if mh >= H // P - 2:
                        # Drain the tail on both rings in halves.
                        hT = t_lo // 2
                        nc.sync.dma_start(
                            y8[mh * P : (mh + 1) * P, :hT], yt[:, :hT]
                        )
                        nc.scalar.dma_start(
                            y8[mh * P : (mh + 1) * P, hT:], yt[:, hT:]
                        )
                    else:
                        rings[mh % 2].dma_start(
                            y8[mh * P : (mh + 1) * P, :], yt[:]
                        )
    nc.compile()
    return nc


def _route(expert_weights, chosen_expert_indices, attention_mask):
    """Host-side routing + w-stratification.

    Returns per-expert (toks_hi, w_hi, toks_lo, w_lo) plus (t_hi, t_lo).
    Pairs are sorted by w ascending within each expert; the l_e =
    c_e - t_hi smallest-w pairs go to the fp8 path.
    """
    idx = np.asarray(chosen_expert_indices).reshape(N, K).astype(np.int64)
    wts = np.asarray(expert_weights).reshape(N, K).astype(np.float32)
    mask = np.asarray(attention_mask).reshape(N, 1).astype(np.float32)
    wts = wts * mask

    flat_e = idx.reshape(-1)
    flat_w = wts.reshape(-1)
    flat_tok = np.repeat(np.arange(N), K)

    order = np.lexsort((flat_w, flat_e))  # expert-major, w ascending
    counts = np.bincount(flat_e, minlength=E)
    offs = np.zeros(E + 1, np.int64)
    np.cumsum(counts, out=offs[1:])
    c_min, c_max = int(counts.min()), int(counts.max())
    w_sorted = flat_w[order]
    w2_total = float((flat_w**2).sum())

    def lo_w2(th):
        s = 0.0
        for e in range(E):
            l = counts[e] - th
            if l > 0:
                seg = w_sorted[offs[e] : offs[e] + l]
                s += float((seg * seg).sum())
        return s

    t_hi = _r8(c_max)  # fallback: all bf16
    lo_cands = range(max(8, _r8(c_max - 512)), c_min + 1, 8)
    for cand in lo_cands:
        if lo_w2(cand) <= FRAC_W2 * max(w2_total, 1e-30):
            t_hi = cand
            break

    per_expert = []
    max_l = 0
    for e in range(E):
        sel = order[offs[e] : offs[e + 1]]
        l = max(0, int(counts[e]) - t_hi)
        max_l = max(max_l, l)
        per_expert.append(
            (
                flat_tok[sel[l:]],
                flat_w[sel[l:]],
                flat_tok[sel[:l]],
                flat_w[sel[:l]],
            )
        )
    t_lo = _r8(max_l)
    return per_expert, t_hi, t_lo


def kernel(x, attention_mask, expert_weights, chosen_expert_indices, down_proj, up_proj):
    global LAST_RESULT
    xt = np.asarray(x, dtype=np.float32).reshape(N, H)
    per_expert, t_hi, t_lo = _route(
        expert_weights, chosen_expert_indices, attention_mask
    )

    xT_full = np.ascontiguousarray(xt.T)  # [H, N]
    down = np.asarray(down_proj, dtype=np.float32)
    up = np.asarray(up_proj, dtype=np.float32)

    in_maps = []
    for e in range(E):
        toks_hi, w_hi, toks_lo, _w_lo = per_expert[e]
        h_e = len(toks_hi)
        xTg = np.zeros((H, t_hi), dtype=BF16)
        # w folded into bf16-path activations (w >= 0, relu positively
        # homogeneous) so that path's output needs no host scaling.
        xTg[:, :h_e] = (xT_full[:, toks_hi] * w_hi[None, :]).astype(BF16)
        m = {
            "xT": xTg,
            "dw": down[e].astype(BF16),
            "up": up[e].astype(BF16),
        }
        if t_lo:
            l_e = len(toks_lo)
            x8g = np.zeros((H, t_lo), dtype=np.float32)
            x8g[:, :l_e] = xT_full[:, toks_lo]  # unweighted for fp8
            m["x8"] = np.ascontiguousarray(
                x8g.astype(F8).reshape(KH2, 2, P, t_lo).transpose(2, 0, 1, 3)
            )
            m["dw8"] = np.ascontiguousarray(
                down[e].astype(F8).reshape(KH2, 2, P, D).transpose(2, 0, 1, 3)
            )
            m["up8"] = np.ascontiguousarray(
                up[e].astype(F8).reshape(KD2, 2, P, H).transpose(2, 0, 1, 3)
            )
        in_maps.append(m)

    nc = _build_bass(t_hi, t_lo)
    # First execution of a freshly loaded NEFF occasionally fails with a
    # transient NRT_EXEC_UNIT_UNRECOVERABLE; a retry has always succeeded.
    last_err = None
    for attempt in range(3):
        try:
            res = run_bass_kernel_spmd(nc, in_maps, core_ids=list(range(NCORES)))
            break
        except Exception as e:  # noqa: BLE001
            last_err = e
            time.sleep(3.0)
    else:
        raise last_err
    LAST_RESULT = res

    acc = xt.copy()
    for e in range(E):
        toks_hi, _w_hi, toks_lo, w_lo = per_expert[e]
        h_e = len(toks_hi)
        y16 = res.results[e]["yT"]  # [H, t_hi] fp32, w pre-folded
        acc[toks_hi] += y16.T[:h_e]
        if t_lo and len(toks_lo):
            l_e = len(toks_lo)
            y8 = res.results[e]["y8"]  # [H, t_lo] fp32, unweighted
            acc[toks_lo] += y8.T[:l_e] * w_lo[:, None]
    return acc.reshape(B, S, H).astype(np.float32)


# revision 14
# speedup vs baseline: 1.0155x; 1.0155x over previous
"""MoE MLP (top-2 of 8 experts) on 8 Trainium2 NeuronCores.

Strategy: expert parallelism with w-stratified mixed precision. Each of
the 8 cores owns one expert. Host-side routing sorts each expert's
(token, k) pairs by routing weight w. The T_HI highest-w pairs per
expert run a dense bf16 pipeline (w folded into the activations); the
remaining low-w pairs run a pure-fp8(e4m3) pipeline using DoubleRow
matmuls (256-deep contraction per instruction = 2x bf16 FLOP rate), w
applied on the host during combine.

Why this is safe: the kernel's error is relative to each contribution's
magnitude, and a pair's contribution is scaled by w. fp8 carries ~5.3%
relative error, so assigning only the smallest-w pairs to fp8 keeps the
global L2 error at ~sqrt(frac_w2) * 5.3% where frac_w2 is the fp8
stratum's share of sum(w^2). T_HI is chosen at runtime as the smallest
count keeping frac_w2 <= 7%, bounding total rel err ~1.5e-2 < 2e-2.

This also load-balances: every core computes exactly T_HI bf16 columns
plus its overflow (c_e - T_HI <= T_LO) fp8 columns, so per-core PE time
is nearly equal without splitting experts across cores.

Device compute per core (fp32 PSUM accumulation):
  bf16: hidT[D,Th] = relu(dw^T @ (w*x)T) ; yT[H,Th] = up^T @ hidT
  fp8:  hid8[D,Tl] = relu8(dw8^T @ x8T)  ; y8T[H,Tl] = up8^T @ hid8
"""

import os
import sys
import time

import numpy as np

for _p in ("/opt/trn_rl_repo", "/root/.axon_site/_ro/trn_rl_repo"):
    if os.path.isdir(_p) and _p not in sys.path:
        sys.path.append(_p)

import ml_dtypes

import concourse.bass as bass
import concourse.mybir as mybir
from concourse import bacc
from concourse.bass_utils import run_bass_kernel_spmd
from concourse.tile import TileContext

BF16 = ml_dtypes.bfloat16
F8 = ml_dtypes.float8_e4m3

B, S, H, E, K, D = 1, 4096, 1024, 8, 2, 2048
N = B * S
P = 128
KH = H // P    # 8 contraction tiles for the bf16 down matmul
KD = D // P    # 16 contraction tiles for the bf16 up matmul
KH2 = KH // 2  # 4 DoubleRow pairs (256-deep) for the fp8 down matmul
KD2 = KD // 2  # 8 DoubleRow pairs for the fp8 up matmul
NCORES = 8

# Max share of sum(w^2) allowed in the fp8 stratum. err ~ 5.3% *
# sqrt(FRAC) + bf16's 0.33% -> ~1.7e-2 vs the 2e-2 gate.
FRAC_W2 = 0.10

# Exposed for test harness introspection (exec_time_ns etc).
LAST_RESULT = None


def _chunks(total: int, maxc: int = 512) -> list[tuple[int, int]]:
    """(offset, size) split of `total` into <=512-wide PSUM chunks.

    The FIRST chunk is made as wide as possible: a wide chunk lowers the
    weight-stream bandwidth the k-outer matmul loop demands per cycle
    (dw bytes/cycle ~ 32768/chunk_width), which is what paces the DMA-
    starved first ~15us of the kernel. A 512+128 split beats even halves
    for 512 < total <= 640."""
    if total <= maxc:
        return [(0, total)]
    if total <= maxc + 128:
        return [(0, total - 128), (total - 128, 128)]
    n = -(-total // maxc)
    base, rem = divmod(total, n)
    out, off = [], 0
    for i in range(n):
        sz = base + (1 if i < rem else 0)
        out.append((off, sz))
        off += sz
    return out


def _r8(v: int) -> int:
    return -(-v // 8) * 8


def _build_bass(t_hi: int, t_lo: int) -> bass.Bass:
    """One expert's mixed-precision MLP. t_lo == 0 disables the fp8 path."""
    bf16 = mybir.dt.bfloat16
    f8 = mybir.dt.float8e4
    f32 = mybir.dt.float32
    DR = mybir.MatmulPerfMode.DoubleRow

    nc = bacc.Bacc()
    xT = nc.dram_tensor("xT", [H, t_hi], bf16, kind="ExternalInput")
    dw = nc.dram_tensor("dw", [H, D], bf16, kind="ExternalInput")
    up = nc.dram_tensor("up", [D, H], bf16, kind="ExternalInput")
    yT = nc.dram_tensor("yT", [H, t_hi], f32, kind="ExternalOutput")
    if t_lo:
        x8 = nc.dram_tensor("x8", [P, KH2, 2, t_lo], f8, kind="ExternalInput")
        dw8 = nc.dram_tensor("dw8", [P, KH2, 2, D], f8, kind="ExternalInput")
        up8 = nc.dram_tensor("up8", [P, KD2, 2, H], f8, kind="ExternalInput")
        y8 = nc.dram_tensor("y8", [H, t_lo], f32, kind="ExternalOutput")

    with TileContext(nc) as tc:
        with (
            tc.tile_pool(name="const", bufs=1) as const,
            tc.tile_pool(name="psum", bufs=1, space="PSUM") as psum,
            tc.tile_pool(name="outp", bufs=4) as outp,
        ):
            dw_sb = const.tile([P, KH, D], bf16)
            xT_sb = const.tile([P, KH, t_hi], bf16)
            up_sb = const.tile([P, KD, H], bf16)
            hid_sb = const.tile([P, KD, t_hi], bf16)
            if t_lo:
                dw8_sb = const.tile([P, KH2, 2, D], f8)
                x8_sb = const.tile([P, KH2, 2, t_lo], f8)
                up8_sb = const.tile([P, KD2, 2, H], f8)
                hid8_sb = const.tile([P, KD2, 2, t_lo], f8)

            # ---- DMA schedule (phases ordered by first-need time) ----
            hD = D // 2
            tchunks = _chunks(t_hi)
            n0_off, n0 = tchunks[0]
            rings = [nc.sync, nc.scalar]
            # Phase 1: dw half A + first token chunk, alternating rings
            # per k so chunk k of both mm1 operands lands together.
            for k in range(KH):
                if k == 0:
                    qD = hD // 2
                    nc.sync.dma_start(dw_sb[:, 0, :qD], dw[:P, :qD])
                    nc.scalar.dma_start(xT_sb[:, 0, :n0], xT[:P, :n0])
                    nc.sync.dma_start(dw_sb[:, 0, qD:hD], dw[:P, qD:hD])
                    continue
                rings[k % 2].dma_start(
                    dw_sb[:, k, :hD], dw[k * P : (k + 1) * P, :hD]
                )
                rings[1 - k % 2].dma_start(
                    xT_sb[:, k, :n0], xT[k * P : (k + 1) * P, :n0]
                )
                # Prefetch half-B k-chunks early so mh1 of the first
                # token chunk never waits on them.
                if k == 3:
                    for kb in (0, 1):
                        rings[kb % 2].dma_start(
                            dw_sb[:, kb, hD:], dw[kb * P : (kb + 1) * P, hD:]
                        )
                if k == 6:
                    for kb in (2, 3):
                        rings[kb % 2].dma_start(
                            dw_sb[:, kb, hD:], dw[kb * P : (kb + 1) * P, hD:]
                        )
            # Phase 2: dw half B.
            for k in range(4, KH):
                rings[k % 2].dma_start(
                    dw_sb[:, k, hD:], dw[k * P : (k + 1) * P, hD:]
                )
            # Phase 3: remaining token chunks.
            for off, sz in tchunks[1:]:
                for k in range(KH):
                    rings[k % 2].dma_start(
                        xT_sb[:, k, off : off + sz],
                        xT[k * P : (k + 1) * P, off : off + sz],
                    )
            # Phase 4: fp8 weights + tokens (needed when mm1lo starts,
            # ~35us in).
            if t_lo:
                for kk in range(KH2):
                    rings[kk % 2].dma_start(x8_sb[:, kk], x8[:, kk])
                for kk in range(KH2):
                    rings[kk % 2].dma_start(
                        dw8_sb[:, kk, :, :hD], dw8[:, kk, :, :hD]
                    )
                    rings[1 - kk % 2].dma_start(
                        dw8_sb[:, kk, :, hD:], dw8[:, kk, :, hD:]
                    )
            # Phase 5: up weights (needed when mm2hi starts, ~47us).
            for k in range(KD):
                rings[k % 2].dma_start(up_sb[:, k, :], up[k * P : (k + 1) * P, :])
            # Phase 6: fp8 up weights (needed when mm2lo starts, ~80us).
            if t_lo:
                for kk in range(KD2):
                    rings[kk % 2].dma_start(up8_sb[:, kk], up8[:, kk])

            # ---- PE warmup (HAM un-throttles after ~3.4us of activity).
            warm_sb = const.tile([P, 640], bf16)
            nc.vector.memset(warm_sb[:], 0.0)
            warm_ps = psum.tile([P, 512], f32, tag="ps0", name="warm_ps")
            n_warm = 13
            for i in range(n_warm):
                nc.tensor.matmul(
                    warm_ps[:],
                    warm_sb[:, :P],
                    warm_sb[:, P:640],
                    start=(i == 0),
                    stop=(i == n_warm - 1),
                )

            # ---- mm1hi: hidT[D,Th] = relu(dw^T @ xT), k-outermost over
            # 8 concurrent PSUM groups.
            for n_off, n_size in tchunks:
                for mh in range(KD // 8):
                    pss = [
                        psum.tile([P, n_size], f32, tag=f"ps{m}", name=f"h{mh}_{m}")
                        for m in range(8)
                    ]
                    for k in range(KH):
                        for m in range(8):
                            md = mh * 8 + m
                            nc.tensor.matmul(
                                pss[m][:],
                                dw_sb[:, k, md * P : (md + 1) * P],
                                xT_sb[:, k, n_off : n_off + n_size],
                                start=(k == 0),
                                stop=(k == KH - 1),
                            )
                    for m in range(8):
                        md = mh * 8 + m
                        nc.vector.tensor_scalar_max(
                            hid_sb[:, md, n_off : n_off + n_size], pss[m][:], 0.0
                        )

            # ---- mm1lo (fp8 DoubleRow): hid8[D,Tl] = relu8(dw8^T@x8).
            # m-outer (weights are resident by now, no DMA pacing need):
            # each PSUM group completes after its 4 DR matmuls and its
            # relu fires immediately, so bank md%8 is free again well
            # before group md+8 needs it — no drain burst at the mh
            # boundary. relu8 lives on VECTOR: the scalar engine is
            # busy issuing DMA ring pushes until ~50us and would stall
            # the whole fp8 phase.
            if t_lo:
                for md in range(KD):
                    ps = psum.tile([P, t_lo], f32, tag=f"ps{md % 8}", name=f"lo{md}")
                    for kk in range(KH2):
                        nc.tensor.matmul(
                            ps[:],
                            dw8_sb[:, kk, :, md * P : (md + 1) * P],
                            x8_sb[:, kk],
                            start=(kk == 0),
                            stop=(kk == KH2 - 1),
                            perf_mode=DR,
                        )
                    nc.vector.tensor_scalar_max(
                        hid8_sb[:, md // 2, md % 2, :], ps[:], 0.0
                    )

            # ---- mm2hi: yT[H,Th] = up^T @ hidT.
            gi = 0
            for mh in range(H // P):
                for n_off, n_size in tchunks:
                    ps = psum.tile([P, n_size], f32, tag=f"ps{gi % 8}")
                    gi += 1
                    for k in range(KD):
                        nc.tensor.matmul(
                            ps[:],
                            up_sb[:, k, mh * P : (mh + 1) * P],
                            hid_sb[:, k, n_off : n_off + n_size],
                            start=(k == 0),
                            stop=(k == KD - 1),
                        )
                    yt = outp.tile([P, n_size], f32, tag="yt")
                    nc.vector.tensor_copy(yt[:], ps[:])
                    rings[gi % 2].dma_start(
                        yT[mh * P : (mh + 1) * P, n_off : n_off + n_size], yt[:]
                    )

            # ---- mm2lo (fp8 DoubleRow): y8T[H,Tl] = up8^T @ hid8.
            if t_lo:
                for mh in range(H // P):
                    ps = psum.tile([P, t_lo], f32, tag=f"ps{mh % 8}")
                    for kk in range(KD2):
                        nc.tensor.matmul(
                            ps[:],
                            up8_sb[:, kk, :, mh * P : (mh + 1) * P],
                            hid8_sb[:, kk],
                            start=(kk == 0),
                            stop=(kk == KD2 - 1),
                            perf_mode=DR,
                        )
                    yt = outp.tile([P, t_lo], f32, tag="yt")
                    nc.vector.tensor_copy(yt[:], ps[:])
                    if mh >= H // P - 2:
                        # Drain the tail on both rings in halves.
                        hT = t_lo // 2
                        nc.sync.dma_start(
                            y8[mh * P : (mh + 1) * P, :hT], yt[:, :hT]
                        )
                        nc.scalar.dma_start(
                            y8[mh * P : (mh + 1) * P, hT:], yt[:, hT:]
                        )
                    else:
                        rings[mh % 2].dma_start(
                            y8[mh * P : (mh + 1) * P, :], yt[:]
                        )
    nc.compile()
    return nc


def _route(expert_weights, chosen_expert_indices, attention_mask):
    """Host-side routing + w-stratification.

    Returns per-expert (toks_hi, w_hi, toks_lo, w_lo) plus (t_hi, t_lo).
    Pairs are sorted by w ascending within each expert; the l_e =
    c_e - t_hi smallest-w pairs go to the fp8 path.
    """
    idx = np.asarray(chosen_expert_indices).reshape(N, K).astype(np.int64)
    wts = np.asarray(expert_weights).reshape(N, K).astype(np.float32)
    mask = np.asarray(attention_mask).reshape(N, 1).astype(np.float32)
    wts = wts * mask

    flat_e = idx.reshape(-1)
    flat_w = wts.reshape(-1)
    flat_tok = np.repeat(np.arange(N), K)

    order = np.lexsort((flat_w, flat_e))  # expert-major, w ascending
    counts = np.bincount(flat_e, minlength=E)
    offs = np.zeros(E + 1, np.int64)
    np.cumsum(counts, out=offs[1:])
    c_min, c_max = int(counts.min()), int(counts.max())
    w_sorted = flat_w[order]
    w2_total = float((flat_w**2).sum())

    def lo_w2(th):
        s = 0.0
        for e in range(E):
            l = counts[e] - th
            if l > 0:
                seg = w_sorted[offs[e] : offs[e] + l]
                s += float((seg * seg).sum())
        return s

    t_hi = _r8(c_max)  # fallback: all bf16
    lo_cands = range(max(8, _r8(c_max - 512)), c_min + 1, 8)
    for cand in lo_cands:
        if lo_w2(cand) <= FRAC_W2 * max(w2_total, 1e-30):
            t_hi = cand
            break

    per_expert = []
    max_l = 0
    for e in range(E):
        sel = order[offs[e] : offs[e + 1]]
        l = max(0, int(counts[e]) - t_hi)
        max_l = max(max_l, l)
        per_expert.append(
            (
                flat_tok[sel[l:]],
                flat_w[sel[l:]],
                flat_tok[sel[:l]],
                flat_w[sel[:l]],
            )
        )
    t_lo = _r8(max_l)
    return per_expert, t_hi, t_lo


def kernel(x, attention_mask, expert_weights, chosen_expert_indices, down_proj, up_proj):
    global LAST_RESULT
    xt = np.asarray(x, dtype=np.float32).reshape(N, H)
    per_expert, t_hi, t_lo = _route(
        expert_weights, chosen_expert_indices, attention_mask
    )

    xT_full = np.ascontiguousarray(xt.T)  # [H, N]
    down = np.asarray(down_proj, dtype=np.float32)
    up = np.asarray(up_proj, dtype=np.float32)

    in_maps = []
    for e in range(E):
        toks_hi, w_hi, toks_lo, _w_lo = per_expert[e]
        h_e = len(toks_hi)
        xTg = np.zeros((H, t_hi), dtype=BF16)
        # w folded into bf16-path activations (w >= 0, relu positively
        # homogeneous) so that path's output needs no host scaling.
        xTg[:, :h_e] = (xT_full[:, toks_hi] * w_hi[None, :]).astype(BF16)
        m = {
            "xT": xTg,
            "dw": down[e].astype(BF16),
            "up": up[e].astype(BF16),
        }
        if t_lo:
            l_e = len(toks_lo)
            x8g = np.zeros((H, t_lo), dtype=np.float32)
            x8g[:, :l_e] = xT_full[:, toks_lo]  # unweighted for fp8
            m["x8"] = np.ascontiguousarray(
                x8g.astype(F8).reshape(KH2, 2, P, t_lo).transpose(2, 0, 1, 3)
            )
            m["dw8"] = np.ascontiguousarray(
                down[e].astype(F8).reshape(KH2, 2, P, D).transpose(2, 0, 1, 3)
            )
            m["up8"] = np.ascontiguousarray(
                up[e].astype(F8).reshape(KD2, 2, P, H).transpose(2, 0, 1, 3)
            )
        in_maps.append(m)

    nc = _build_bass(t_hi, t_lo)
    # First execution of a freshly loaded NEFF occasionally fails with a
    # transient NRT_EXEC_UNIT_UNRECOVERABLE; a retry has always succeeded.
    last_err = None
    for attempt in range(3):
        try:
            res = run_bass_kernel_spmd(nc, in_maps, core_ids=list(range(NCORES)))
            break
        except Exception as e:  # noqa: BLE001
            last_err = e
            time.sleep(3.0)
    else:
        raise last_err
    LAST_RESULT = res

    acc = xt.copy()
    for e in range(E):
        toks_hi, _w_hi, toks_lo, w_lo = per_expert[e]
        h_e = len(toks_hi)
        y16 = res.results[e]["yT"]  # [H, t_hi] fp32, w pre-folded
        acc[toks_hi] += y16.T[:h_e]
        if t_lo and len(toks_lo):
            l_e = len(toks_lo)
            y8 = res.results[e]["y8"]  # [H, t_lo] fp32, unweighted
            acc[toks_lo] += y8.T[:l_e] * w_lo[:, None]
    return acc.reshape(B, S, H).astype(np.float32)
